# revision 39
# baseline (speedup 1.0000x reference)
"""Sparse (shot-local + shared-global) attention on 8 Trainium2 NeuronCores.

Problem: B=2, S_TOT=4096, HD=1024 with H=16 heads (d=64), num_shots=4
(L=1024 tokens per shot), global pool = first 64 tokens of each shot
(G=256), shared by all shots of the same batch element.

Sharding: the 32 (batch, head) pairs are split 4-per-core across 8 cores
(data + head parallel). Each (b,h,shot) block is independent attention of
shape q[1024,64] against k/v[1024+256,64].

Per-core kernel, one linear stream of 320 (pair, shot, q-chunk, k-slot)
steps (10 slots per unit = 8 local k tiles + 2 global tiles):
  S^T[k,q] = kT_tile.T @ qT          (PE, into 2x 3-bank PSUM windows)
  P^T      = exp(S^T * 1/8)          (ACT, one ACTIVATE per 3-bank window)
  [o^T; Z] = [v | 1].T @ P^T         (PE, accumulated over the 10 slots)
  DVE copies [o^T; Z] PSUM->SBUF fp16; DMA ships it out unnormalized.
The o/Z division happens on the host during unshard.

TRN2 scheduling specifics this kernel is built around:
 - HAM clock gate: PE runs at 2.4GHz only while its MAC stream is gapless
   over 3.4us windows, else 1.2GHz. So every matmul is shaped 128x128xN
   (q/k contraction zero-padded 64->128, v weight cols zero-padded
   65->128) so LDWEIGHTS always pairs with the background weight slot and
   hides under the previous matmul; and idempotent duplicate QK matmuls
   (same dst, same operands, start=True) pad each window so PE work per
   window slightly exceeds ACT work - the PE never idles.
 - Tile dependency tracking: the two PSUM window tensors are separate
   tiles so a window-w QK write only waits on the window-(w-2) ACTIVATE.
 - The normalize (recip+broadcast+mul) is entirely off-device: it would
   otherwise put GpSimd/DVE in the PE<->ACT critical loop.
Softmax max-subtraction is skipped: logits are ~N(0,1), |logit| < ~7,
exp is safely in fp16 range. Matmul operands fp16, PSUM fp32.
"""

import sys

sys.path.insert(0, "/opt/trn_rl_repo")

import ml_dtypes
import numpy as np

import concourse.bass as bass  # noqa: F401  (registers AP machinery)
import concourse.mybir as mybir
import concourse.tile as tile
from concourse import bacc
from concourse.bass_utils import run_bass_kernel_spmd

B, S_TOT, HD = 2, 4096, 1024
H, NSHOT, PER_G = 16, 4, 64
D = HD // H            # 64 head dim
L = S_TOT // NSHOT     # 1024 shot length
G = NSHOT * PER_G      # 256 global pool tokens
NCORES = 8
PAIRS = (B * H) // NCORES   # 4 (b,h) pairs per core
QC = 512                    # q chunk width (PSUM bank)
NQC = L // QC               # 2
NKT_LOC = L // 128          # 8 local k tiles per shot
NKT = NKT_LOC + G // 128    # 10 k tiles (slots) total per shot
SCALE = 1.0 / float(np.sqrt(D))
WIN = 3                     # slots per ACTIVATE window (3 banks = 1536 cols)
FILL = [512, 128]           # widths of idempotent dup-QK fillers (odd windows
                            # add FILL[1] so mean PE work/window == ACT work)
NSTEP = PAIRS * NSHOT * NQC * NKT   # 320

MM_DT = "float16"

_NC = None


def build_program():
    """Build + compile the per-core Bass program (identical on all cores)."""
    global _NC
    if _NC is not None:
        return _NC
    f32 = mybir.dt.float32
    mdt = getattr(mybir.dt, MM_DT)
    Exp = mybir.ActivationFunctionType.Exp

    nc = bacc.Bacc("TRN2", target_bir_lowering=False, debug=True)
    # q/k shipped with zero rows 64..127 (contraction padding) and v with
    # zero cols 65..127 (weight-column padding): every matmul is then a
    # uniform 128x128 stationary tile, which keeps LDWEIGHTS hidden.
    qT_d = nc.dram_tensor("qT", [D, PAIRS, S_TOT], mdt, kind="ExternalInput")
    kT_d = nc.dram_tensor("kT", [D, PAIRS, S_TOT], mdt, kind="ExternalInput")
    kgT_d = nc.dram_tensor("kgT", [128, PAIRS, G], mdt, kind="ExternalInput")
    v128_d = nc.dram_tensor("v128", [128, PAIRS, NKT_LOC * NSHOT, 128], mdt,
                            kind="ExternalInput")
    vg128_d = nc.dram_tensor("vg128", [128, PAIRS, G // 128, 128], mdt,
                             kind="ExternalInput")
    oT_d = nc.dram_tensor("oT", [65, PAIRS, S_TOT], mdt, kind="ExternalOutput")
    fence_d = nc.dram_tensor("fence", [1, 16], mdt, kind="ExternalOutput")

    # step index -> (pair, shot, qchunk, slot)
    STEP = [(p, s, qc, j)
            for p in range(PAIRS)
            for s in range(NSHOT)
            for qc in range(NQC)
            for j in range(NKT)]
    # Window boundaries: the first window is 2 slots (same total ACTIVATE
    # count, but the first exp fires one matmul earlier), then 3-slot
    # windows; 2 + 106*3 = 320 exactly.
    WB = [0, 2] + list(range(5, NSTEP + 1, WIN))
    NWIN = len(WB) - 1
    S2W = {}
    for w in range(NWIN):
        for i in range(WB[w], WB[w + 1]):
            S2W[i] = w

    with tile.TileContext(nc) as tc:
        with (
            tc.tile_pool(name="inp", bufs=2) as inp_pool,
            tc.tile_pool(name="expp", bufs=4) as exp_pool,
            tc.tile_pool(name="ob", bufs=3) as ob_pool,
            tc.tile_pool(name="warm", bufs=1) as warm_pool,
            tc.tile_pool(name="ps_s", bufs=1, space="PSUM") as ps_pool,
            tc.tile_pool(name="ps_o", bufs=2, space="PSUM") as po_pool,
        ):
            # Two independent 3-bank window tensors (alternating windows).
            psW = [ps_pool.tile([128, WIN * QC], f32, tag=t, name=t)
                   for t in ("psA", "psB")]

            def load_pair(p):
                # One tile PER SHOT for q/k/v: Tile merges dependencies per
                # tensor, so a shared whole-pair tile would make the first
                # QK wait for every shot's DMA. q/k ship unpadded (64 rows);
                # SBUF rows 64..127 are zeroed once per physical buffer
                # (pairs 0/1 only - DMA never touches them after, so they
                # stay zero across buffer reuse). The zeroing runs on DVE
                # for the startup-critical pair-0 shot-0 tiles and on the
                # otherwise-idle GpSimd for the rest. Shot-0 q/k DMAs are
                # split so the first QK starts after ~100KB of transfer.
                # q is tiled per (shot, chunk) and shot-0 k gets a 4-slot
                # head tile: dependencies merge per tensor, so the first QKs
                # must not share a tensor with DMAs they don't read. The DMA
                # engines round-robin across queued descriptors (not FIFO),
                # so for pair 0 only the head transfers are queued up front;
                # the bulk is staged across the first windows in need order
                # (returned as closures) so the head completes in ~0.3us.
                qT = [[inp_pool.tile([128, QC], mdt, tag=f"qT{s}{qc}",
                                     name=f"qT{s}{qc}") for qc in range(NQC)]
                      for s in range(NSHOT)]
                kT = [inp_pool.tile([128, L], mdt, tag=f"kT{s}", name=f"kT{s}")
                      for s in range(NSHOT)]
                k0h = inp_pool.tile([128, 640], mdt, tag="k0h", name="k0h")
                vv = [inp_pool.tile([128, NKT_LOC, 128], mdt, tag=f"v{s}",
                                    name=f"v{s}") for s in range(NSHOT)]
                kgT_sb = inp_pool.tile([128, G], mdt, tag="kgT", name="kgT_sb")
                vg_sb = inp_pool.tile([128, G // 128, 128], mdt, tag="vg128",
                                      name="vg_sb")
                if p < 2:
                    crit = nc.vector if p == 0 else nc.gpsimd
                    crit.memset(k0h[D:, :], 0.0)
                    crit.memset(qT[0][0][D:, :], 0.0)
                    crit.memset(qT[0][1][D:, :], 0.0)
                    crit.memset(kT[0][D:, :], 0.0)
                    for s in range(1, NSHOT):
                        for qc in range(NQC):
                            nc.gpsimd.memset(qT[s][qc][D:, :], 0.0)
                        nc.gpsimd.memset(kT[s][D:, :], 0.0)
                nc.sync.dma_start(qT[0][0][:D, :], qT_d[:, p, :QC])
                nc.sync.dma_start(k0h[:D, :], kT_d[:, p, :640])

                def shot_stage(s):
                    def emit():
                        for qc in range(NQC):
                            nc.sync.dma_start(qT[s][qc][:D, :],
                                              qT_d[:, p, s * L + qc * QC:
                                                   s * L + (qc + 1) * QC])
                        nc.sync.dma_start(kT[s][:D, :],
                                          kT_d[:, p, s * L:(s + 1) * L])
                        nc.sync.dma_start(vv[s][:],
                                          v128_d[:, p,
                                                 s * NKT_LOC:(s + 1) * NKT_LOC, :])
                    return emit

                def stage_a():
                    nc.sync.dma_start(kT[0][:D, :], kT_d[:, p, :L])
                    nc.sync.dma_start(qT[0][1][:D, :], qT_d[:, p, QC:L])
                    nc.sync.dma_start(kgT_sb[:], kgT_d[:, p, :])

                def stage_b():
                    nc.sync.dma_start(vg_sb[:], vg128_d[:, p, :, :])
                    nc.sync.dma_start(vv[0][:], v128_d[:, p, :NKT_LOC, :])

                stages = [stage_a, stage_b] + [shot_stage(s)
                                               for s in range(1, NSHOT)]
                sb = {"qT": qT, "kT": kT, "k0h": k0h, "kgT": kgT_sb,
                      "v": vv, "vg": vg_sb}
                return sb, stages

            # Warmup: a garbage exp hides the ACT table load and a few dummy
            # matmuls start the HAM activity window while input DMAs land.
            # garb memset runs on GpSimd, which starts ~1.3us before the
            # other engines in the NEFF preamble - the warmup matmuls can
            # then issue the moment the tensor engine comes up.
            garb = warm_pool.tile([128, 640], mdt, tag="garb", name="garb")
            nc.gpsimd.memset(garb[:], 0.0)
            wexp = warm_pool.tile([128, 32], mdt, tag="wexp", name="wexp")
            nc.scalar.activation(wexp[:], garb[:, :32], Exp, scale=1.0)
            for _ in range(3):
                nc.tensor.matmul(psW[0][:, :QC], garb[:, :128], garb[:, 128:],
                                 start=True, stop=True)

            # DMA fence: the Sync queue is in-order, so a dummy DMA whose
            # source is only written by a GpSimd memset (finishing ~1.3us
            # after the head transfers are queued) holds ALL later input
            # descriptors back - the DMA engines round-robin across queued
            # descriptors, and without this the first QK's 144KB completes
            # only after megabytes of co-queued bulk.
            fence_sb = warm_pool.tile([128, 512], mdt, tag="fence",
                                      name="fence_sb")
            nc.gpsimd.memset(fence_sb[:], 0.0)

            SB = {}
            SB[0], stages0 = load_pair(0)
            nc.sync.dma_start(fence_d[:], fence_sb[:1, :16])
            STAGE_AT = {0: [stages0[0]], 1: [stages0[1], stages0[2]],
                        2: [stages0[3]], 3: [stages0[4]]}
            EX = {}      # window -> expT tile
            PO = [None]  # current unit's po accumulator

            def QK(i, dup_cols=None):
                p, s, qc, j = STEP[i]
                sb = SB[p]
                w = S2W[i]
                off = i - WB[w]
                if j < 5 and s == 0:
                    k_lhs = sb["k0h"][:, j * 128:(j + 1) * 128]
                elif j < NKT_LOC:
                    k_lhs = sb["kT"][s][:, j * 128:(j + 1) * 128]
                else:
                    gg = j - NKT_LOC
                    k_lhs = sb["kgT"][:, gg * 128:(gg + 1) * 128]
                n = QC if dup_cols is None else dup_cols
                nc.tensor.matmul(psW[w % 2][:, off * QC: off * QC + n],
                                 k_lhs, sb["qT"][s][qc][:, :n],
                                 start=True, stop=True)

            def ACT(w):
                n = WB[w + 1] - WB[w]
                expT = exp_pool.tile([128, WIN * QC], mdt, tag="expT",
                                     name="expT")
                nc.scalar.activation(expT[:, :n * QC],
                                     psW[w % 2][:, :n * QC],
                                     Exp, scale=SCALE)
                EX[w] = expT

            def PV(i):
                p, s, qc, j = STEP[i]
                sb = SB[p]
                if j == 0:
                    PO[0] = po_pool.tile([128, QC], f32, tag="po", name="po")
                po = PO[0]
                w = S2W[i]
                off = i - WB[w]
                expT = EX[w]
                if j < NKT_LOC:
                    v_lhs = sb["v"][s][:, j, :]
                else:
                    v_lhs = sb["vg"][:, j - NKT_LOC, :]
                nc.tensor.matmul(po[:], v_lhs, expT[:, off * QC:(off + 1) * QC],
                                 start=(j == 0), stop=(j == NKT - 1))
                if j == NKT - 1:
                    ob = ob_pool.tile([65, QC], mdt, tag="ob", name="ob")
                    nc.vector.tensor_copy(ob[:], po[:65, :])
                    qcol = s * L + qc * QC
                    nc.sync.dma_start(oT_d[:, p, qcol:qcol + QC], ob[:])

            # Window-grouped emission: runs of same-shape matmuls keep
            # LDWEIGHTS in the background weight slot (hidden); the dup-QK
            # fillers (idempotent rewrites of the window's first columns)
            # top up PE work per window to just above the ACTIVATE time so
            # the PE stream never idles and HAM holds the 2.4GHz clock.
            STEPS_PER_PAIR = NSHOT * NQC * NKT   # 80
            for w in range(NWIN + 2):
                if w < NWIN:
                    for f in STAGE_AT.pop(w, []):
                        f()
                    # Mean PE work per window matches ACT exactly: the
                    # 2-window PSUM lead is an elastic buffer (~3us) that a
                    # zero-mean deficit never drains over the 107 windows.
                    fill = FILL if w % 2 else FILL[:1]
                    first = True
                    for i in range(WB[w], WB[w + 1]):
                        if i % STEPS_PER_PAIR == 48:
                            nxt = i // STEPS_PER_PAIR + 1
                            if nxt < PAIRS:
                                SB[nxt], st = load_pair(nxt)
                                for f in st:
                                    f()
                        QK(i)
                        if first:
                            for fc in fill:
                                QK(i, dup_cols=fc)
                            first = False
                    ACT(w)
                v = w - 2
                if v >= 0:
                    for i in range(v * WIN, min((v + 1) * WIN, NSTEP)):
                        PV(i)
    nc.compile()
    _NC = nc
    return nc


def pack_inputs(q, k, v):
    """Shard + relayout full inputs into per-core input maps."""
    ndt = ml_dtypes.bfloat16 if MM_DT == "bfloat16" else np.float16
    q5 = np.ascontiguousarray(q).reshape(B, S_TOT, H, D)
    k5 = np.ascontiguousarray(k).reshape(B, S_TOT, H, D)
    v5 = np.ascontiguousarray(v).reshape(B, S_TOT, H, D)
    gidx = (np.arange(NSHOT)[:, None] * L + np.arange(PER_G)[None, :]).reshape(-1)

    in_maps = []
    for c in range(NCORES):
        qT = np.zeros((D, PAIRS, S_TOT), ndt)
        kT = np.zeros((D, PAIRS, S_TOT), ndt)
        kgT = np.zeros((128, PAIRS, G), ndt)
        v128 = np.zeros((128, PAIRS, NKT_LOC * NSHOT, 128), ndt)
        vg128 = np.zeros((128, PAIRS, G // 128, 128), ndt)
        v128[:, :, :, 64] = 1
        vg128[:, :, :, 64] = 1
        for p in range(PAIRS):
            pair = c * PAIRS + p
            b, h = divmod(pair, H)
            qT[:, p, :] = q5[b, :, h, :].T
            kT[:, p, :] = k5[b, :, h, :].T
            kgT[:D, p, :] = k5[b, gidx, h, :].T
            # [S_TOT, 64] -> [n_tiles, 128, 64] -> [128, n_tiles, 64]
            v128[:, p, :, :64] = v5[b, :, h, :].reshape(-1, 128, D).transpose(1, 0, 2)
            vg128[:, p, :, :64] = v5[b, gidx, h, :].reshape(-1, 128, D).transpose(1, 0, 2)
        in_maps.append({"qT": qT, "kT": kT, "kgT": kgT,
                        "v128": v128, "vg128": vg128})
    return in_maps


def unpack_outputs(results):
    """Per-core oT [65, PAIRS, S_TOT] (fp16, unnormalized + Z row) ->
    full [B, S_TOT, HD] fp32, dividing by Z on the host."""
    out5 = np.empty((B, S_TOT, H, D), np.float32)
    for c in range(NCORES):
        oT = np.asarray(results[c]["oT"], dtype=np.float32)
        for p in range(PAIRS):
            b, h = divmod(c * PAIRS + p, H)
            out5[b, :, h, :] = (oT[:64, p, :] / oT[64:65, p, :]).T
    return out5.reshape(B, S_TOT, HD)


def kernel(q, k, v, num_heads, num_shots, per_g):
    assert int(num_heads) == H and int(num_shots) == NSHOT and int(per_g) == PER_G
    nc = build_program()
    in_maps = pack_inputs(np.asarray(q), np.asarray(k), np.asarray(v))
    res = run_bass_kernel_spmd(nc, in_maps, list(range(NCORES)))
    return unpack_outputs(res.results)


# revision 40
# speedup vs baseline: 1.0012x; 1.0012x over previous
"""Sparse (shot-local + shared-global) attention on 8 Trainium2 NeuronCores.

Problem: B=2, S_TOT=4096, HD=1024 with H=16 heads (d=64), num_shots=4
(L=1024 tokens per shot), global pool = first 64 tokens of each shot
(G=256), shared by all shots of the same batch element.

Sharding: the 32 (batch, head) pairs are split 4-per-core across 8 cores
(data + head parallel). Each (b,h,shot) block is independent attention of
shape q[1024,64] against k/v[1024+256,64].

Per-core kernel, one linear stream of 320 (pair, shot, q-chunk, k-slot)
steps (10 slots per unit = 8 local k tiles + 2 global tiles):
  S^T[k,q] = kT_tile.T @ qT          (PE, into 2x 3-bank PSUM windows)
  P^T      = exp(S^T * 1/8)          (ACT, one ACTIVATE per 3-bank window)
  [o^T; Z] = [v | 1].T @ P^T         (PE, accumulated over the 10 slots)
  DVE copies [o^T; Z] PSUM->SBUF fp16; DMA ships it out unnormalized.
The o/Z division happens on the host during unshard.

TRN2 scheduling specifics this kernel is built around:
 - HAM clock gate: PE runs at 2.4GHz only while its MAC stream is gapless
   over 3.4us windows, else 1.2GHz. So every matmul is shaped 128x128xN
   (q/k contraction zero-padded 64->128, v weight cols zero-padded
   65->128) so LDWEIGHTS always pairs with the background weight slot and
   hides under the previous matmul; and idempotent duplicate QK matmuls
   (same dst, same operands, start=True) pad each window so PE work per
   window slightly exceeds ACT work - the PE never idles.
 - Tile dependency tracking: the two PSUM window tensors are separate
   tiles so a window-w QK write only waits on the window-(w-2) ACTIVATE.
 - The normalize (recip+broadcast+mul) is entirely off-device: it would
   otherwise put GpSimd/DVE in the PE<->ACT critical loop.
Softmax max-subtraction is skipped: logits are ~N(0,1), |logit| < ~7,
exp is safely in fp16 range. Matmul operands fp16, PSUM fp32.
"""

import sys

sys.path.insert(0, "/opt/trn_rl_repo")

import ml_dtypes
import numpy as np

import concourse.bass as bass  # noqa: F401  (registers AP machinery)
import concourse.mybir as mybir
import concourse.tile as tile
from concourse import bacc
from concourse.bass_utils import run_bass_kernel_spmd

B, S_TOT, HD = 2, 4096, 1024
H, NSHOT, PER_G = 16, 4, 64
D = HD // H            # 64 head dim
L = S_TOT // NSHOT     # 1024 shot length
G = NSHOT * PER_G      # 256 global pool tokens
NCORES = 8
PAIRS = (B * H) // NCORES   # 4 (b,h) pairs per core
QC = 512                    # q chunk width (PSUM bank)
NQC = L // QC               # 2
NKT_LOC = L // 128          # 8 local k tiles per shot
NKT = NKT_LOC + G // 128    # 10 k tiles (slots) total per shot
SCALE = 1.0 / float(np.sqrt(D))
WIN = 3                     # slots per ACTIVATE window (3 banks = 1536 cols)
FILL = [512, 128]           # widths of idempotent dup-QK fillers (odd windows
                            # add FILL[1] so mean PE work/window == ACT work)
NSTEP = PAIRS * NSHOT * NQC * NKT   # 320

MM_DT = "float16"

_NC = None


def build_program():
    """Build + compile the per-core Bass program (identical on all cores)."""
    global _NC
    if _NC is not None:
        return _NC
    f32 = mybir.dt.float32
    mdt = getattr(mybir.dt, MM_DT)
    Exp = mybir.ActivationFunctionType.Exp

    nc = bacc.Bacc("TRN2", target_bir_lowering=False, debug=True)
    # q/k shipped with zero rows 64..127 (contraction padding) and v with
    # zero cols 65..127 (weight-column padding): every matmul is then a
    # uniform 128x128 stationary tile, which keeps LDWEIGHTS hidden.
    qT_d = nc.dram_tensor("qT", [D, PAIRS, S_TOT], mdt, kind="ExternalInput")
    kT_d = nc.dram_tensor("kT", [D, PAIRS, S_TOT], mdt, kind="ExternalInput")
    kgT_d = nc.dram_tensor("kgT", [128, PAIRS, G], mdt, kind="ExternalInput")
    v65_d = nc.dram_tensor("v65", [128, PAIRS, NKT_LOC * NSHOT, 65], mdt,
                           kind="ExternalInput")
    vg65_d = nc.dram_tensor("vg65", [128, PAIRS, G // 128, 65], mdt,
                            kind="ExternalInput")
    oT_d = nc.dram_tensor("oT", [65, PAIRS, S_TOT], mdt, kind="ExternalOutput")
    fence_d = nc.dram_tensor("fence", [1, 16], mdt, kind="ExternalOutput")

    # step index -> (pair, shot, qchunk, slot)
    STEP = [(p, s, qc, j)
            for p in range(PAIRS)
            for s in range(NSHOT)
            for qc in range(NQC)
            for j in range(NKT)]
    # Window boundaries: the first window is 2 slots (same total ACTIVATE
    # count, but the first exp fires one matmul earlier), then 3-slot
    # windows; 2 + 106*3 = 320 exactly.
    WB = [0, 2] + list(range(5, NSTEP + 1, WIN))
    NWIN = len(WB) - 1
    S2W = {}
    for w in range(NWIN):
        for i in range(WB[w], WB[w + 1]):
            S2W[i] = w

    with tile.TileContext(nc) as tc:
        with (
            tc.tile_pool(name="inp", bufs=2) as inp_pool,
            tc.tile_pool(name="expp", bufs=4) as exp_pool,
            tc.tile_pool(name="ob", bufs=3) as ob_pool,
            tc.tile_pool(name="warm", bufs=1) as warm_pool,
            tc.tile_pool(name="ps_s", bufs=1, space="PSUM") as ps_pool,
            tc.tile_pool(name="ps_o", bufs=2, space="PSUM") as po_pool,
        ):
            # Two independent 3-bank window tensors (alternating windows).
            psW = [ps_pool.tile([128, WIN * QC], f32, tag=t, name=t)
                   for t in ("psA", "psB")]

            def load_pair(p):
                # One tile PER SHOT for q/k/v: Tile merges dependencies per
                # tensor, so a shared whole-pair tile would make the first
                # QK wait for every shot's DMA. q/k ship unpadded (64 rows);
                # SBUF rows 64..127 are zeroed once per physical buffer
                # (pairs 0/1 only - DMA never touches them after, so they
                # stay zero across buffer reuse). The zeroing runs on DVE
                # for the startup-critical pair-0 shot-0 tiles and on the
                # otherwise-idle GpSimd for the rest. Shot-0 q/k DMAs are
                # split so the first QK starts after ~100KB of transfer.
                # q is tiled per (shot, chunk) and shot-0 k gets a 4-slot
                # head tile: dependencies merge per tensor, so the first QKs
                # must not share a tensor with DMAs they don't read. The DMA
                # engines round-robin across queued descriptors (not FIFO),
                # so for pair 0 only the head transfers are queued up front;
                # the bulk is staged across the first windows in need order
                # (returned as closures) so the head completes in ~0.3us.
                qT = [[inp_pool.tile([128, QC], mdt, tag=f"qT{s}{qc}",
                                     name=f"qT{s}{qc}") for qc in range(NQC)]
                      for s in range(NSHOT)]
                kT = [inp_pool.tile([128, L], mdt, tag=f"kT{s}", name=f"kT{s}")
                      for s in range(NSHOT)]
                k0h = inp_pool.tile([128, 640], mdt, tag="k0h", name="k0h")
                vv = [inp_pool.tile([128, NKT_LOC, 128], mdt, tag=f"v{s}",
                                    name=f"v{s}") for s in range(NSHOT)]
                kgT_sb = inp_pool.tile([128, G], mdt, tag="kgT", name="kgT_sb")
                vg_sb = inp_pool.tile([128, G // 128, 128], mdt, tag="vg128",
                                      name="vg_sb")
                if p < 2:
                    crit = nc.vector if p == 0 else nc.gpsimd
                    crit.memset(k0h[D:, :], 0.0)
                    crit.memset(qT[0][0][D:, :], 0.0)
                    crit.memset(qT[0][1][D:, :], 0.0)
                    crit.memset(kT[0][D:, :], 0.0)
                    for s in range(1, NSHOT):
                        for qc in range(NQC):
                            nc.gpsimd.memset(qT[s][qc][D:, :], 0.0)
                        nc.gpsimd.memset(kT[s][D:, :], 0.0)
                nc.sync.dma_start(qT[0][0][:D, :], qT_d[:, p, :QC])
                nc.sync.dma_start(k0h[:D, :], kT_d[:, p, :640])

                def shot_stage(s):
                    def emit():
                        for qc in range(NQC):
                            nc.sync.dma_start(qT[s][qc][:D, :],
                                              qT_d[:, p, s * L + qc * QC:
                                                   s * L + (qc + 1) * QC])
                        nc.sync.dma_start(kT[s][:D, :],
                                          kT_d[:, p, s * L:(s + 1) * L])
                        nc.sync.dma_start(vv[s][:, :, :65],
                                          v65_d[:, p,
                                                s * NKT_LOC:(s + 1) * NKT_LOC, :])
                    return emit

                def stage_a():
                    nc.sync.dma_start(kT[0][:D, :], kT_d[:, p, :L])
                    nc.sync.dma_start(qT[0][1][:D, :], qT_d[:, p, QC:L])
                    nc.sync.dma_start(kgT_sb[:], kgT_d[:, p, :])

                def stage_b():
                    nc.sync.dma_start(vg_sb[:, :, :65], vg65_d[:, p, :, :])
                    nc.sync.dma_start(vv[0][:, :, :65], v65_d[:, p, :NKT_LOC, :])

                stages = [stage_a, stage_b] + [shot_stage(s)
                                               for s in range(1, NSHOT)]
                sb = {"qT": qT, "kT": kT, "k0h": k0h, "kgT": kgT_sb,
                      "v": vv, "vg": vg_sb}
                return sb, stages

            # Warmup: a garbage exp hides the ACT table load and a few dummy
            # matmuls start the HAM activity window while input DMAs land.
            # garb memset runs on GpSimd, which starts ~1.3us before the
            # other engines in the NEFF preamble - the warmup matmuls can
            # then issue the moment the tensor engine comes up.
            garb = warm_pool.tile([128, 640], mdt, tag="garb", name="garb")
            nc.gpsimd.memset(garb[:], 0.0)
            wexp = warm_pool.tile([128, 32], mdt, tag="wexp", name="wexp")
            nc.scalar.activation(wexp[:], garb[:, :32], Exp, scale=1.0)
            for _ in range(3):
                nc.tensor.matmul(psW[0][:, :QC], garb[:, :128], garb[:, 128:],
                                 start=True, stop=True)

            # DMA fence: the Sync queue is in-order, so a dummy DMA whose
            # source is only written by a GpSimd memset (finishing ~1.3us
            # after the head transfers are queued) holds ALL later input
            # descriptors back - the DMA engines round-robin across queued
            # descriptors, and without this the first QK's 144KB completes
            # only after megabytes of co-queued bulk.
            fence_sb = warm_pool.tile([128, 512], mdt, tag="fence",
                                      name="fence_sb")
            nc.gpsimd.memset(fence_sb[:], 0.0)

            SB = {}
            SB[0], stages0 = load_pair(0)
            nc.sync.dma_start(fence_d[:], fence_sb[:1, :16])
            STAGE_AT = {0: [stages0[0]], 1: [stages0[1], stages0[2]],
                        2: [stages0[3]], 3: [stages0[4]]}
            EX = {}      # window -> expT tile
            PO = [None]  # current unit's po accumulator

            def QK(i, dup_cols=None):
                p, s, qc, j = STEP[i]
                sb = SB[p]
                w = S2W[i]
                off = i - WB[w]
                if j < 5 and s == 0:
                    k_lhs = sb["k0h"][:, j * 128:(j + 1) * 128]
                elif j < NKT_LOC:
                    k_lhs = sb["kT"][s][:, j * 128:(j + 1) * 128]
                else:
                    gg = j - NKT_LOC
                    k_lhs = sb["kgT"][:, gg * 128:(gg + 1) * 128]
                n = QC if dup_cols is None else dup_cols
                nc.tensor.matmul(psW[w % 2][:, off * QC: off * QC + n],
                                 k_lhs, sb["qT"][s][qc][:, :n],
                                 start=True, stop=True)

            def ACT(w):
                n = WB[w + 1] - WB[w]
                expT = exp_pool.tile([128, WIN * QC], mdt, tag="expT",
                                     name="expT")
                nc.scalar.activation(expT[:, :n * QC],
                                     psW[w % 2][:, :n * QC],
                                     Exp, scale=SCALE)
                EX[w] = expT

            def PV(i):
                p, s, qc, j = STEP[i]
                sb = SB[p]
                if j == 0:
                    PO[0] = po_pool.tile([128, QC], f32, tag="po", name="po")
                po = PO[0]
                w = S2W[i]
                off = i - WB[w]
                expT = EX[w]
                if j < NKT_LOC:
                    v_lhs = sb["v"][s][:, j, :]
                else:
                    v_lhs = sb["vg"][:, j - NKT_LOC, :]
                nc.tensor.matmul(po[:], v_lhs, expT[:, off * QC:(off + 1) * QC],
                                 start=(j == 0), stop=(j == NKT - 1))
                if j == NKT - 1:
                    ob = ob_pool.tile([65, QC], mdt, tag="ob", name="ob")
                    nc.vector.tensor_copy(ob[:], po[:65, :])
                    qcol = s * L + qc * QC
                    nc.sync.dma_start(oT_d[:, p, qcol:qcol + QC], ob[:])

            # Window-grouped emission: runs of same-shape matmuls keep
            # LDWEIGHTS in the background weight slot (hidden); the dup-QK
            # fillers (idempotent rewrites of the window's first columns)
            # top up PE work per window to just above the ACTIVATE time so
            # the PE stream never idles and HAM holds the 2.4GHz clock.
            STEPS_PER_PAIR = NSHOT * NQC * NKT   # 80
            for w in range(NWIN + 2):
                if w < NWIN:
                    for f in STAGE_AT.pop(w, []):
                        f()
                    # Mean PE work per window matches ACT exactly: the
                    # 2-window PSUM lead is an elastic buffer (~3us) that a
                    # zero-mean deficit never drains over the 107 windows.
                    fill = FILL if w % 2 else FILL[:1]
                    first = True
                    for i in range(WB[w], WB[w + 1]):
                        if i % STEPS_PER_PAIR == 48:
                            nxt = i // STEPS_PER_PAIR + 1
                            if nxt < PAIRS:
                                SB[nxt], st = load_pair(nxt)
                                for f in st:
                                    f()
                        QK(i)
                        if first:
                            for fc in fill:
                                QK(i, dup_cols=fc)
                            first = False
                    ACT(w)
                v = w - 2
                if v >= 0:
                    for i in range(v * WIN, min((v + 1) * WIN, NSTEP)):
                        PV(i)
    nc.compile()
    _NC = nc
    return nc


def pack_inputs(q, k, v):
    """Shard + relayout full inputs into per-core input maps."""
    ndt = ml_dtypes.bfloat16 if MM_DT == "bfloat16" else np.float16
    q5 = np.ascontiguousarray(q).reshape(B, S_TOT, H, D)
    k5 = np.ascontiguousarray(k).reshape(B, S_TOT, H, D)
    v5 = np.ascontiguousarray(v).reshape(B, S_TOT, H, D)
    gidx = (np.arange(NSHOT)[:, None] * L + np.arange(PER_G)[None, :]).reshape(-1)

    in_maps = []
    for c in range(NCORES):
        qT = np.zeros((D, PAIRS, S_TOT), ndt)
        kT = np.zeros((D, PAIRS, S_TOT), ndt)
        kgT = np.zeros((128, PAIRS, G), ndt)
        v65 = np.ones((128, PAIRS, NKT_LOC * NSHOT, 65), ndt)
        vg65 = np.ones((128, PAIRS, G // 128, 65), ndt)
        for p in range(PAIRS):
            pair = c * PAIRS + p
            b, h = divmod(pair, H)
            qT[:, p, :] = q5[b, :, h, :].T
            kT[:, p, :] = k5[b, :, h, :].T
            kgT[:D, p, :] = k5[b, gidx, h, :].T
            # [S_TOT, 64] -> [n_tiles, 128, 64] -> [128, n_tiles, 64]
            v65[:, p, :, :64] = v5[b, :, h, :].reshape(-1, 128, D).transpose(1, 0, 2)
            vg65[:, p, :, :64] = v5[b, gidx, h, :].reshape(-1, 128, D).transpose(1, 0, 2)
        in_maps.append({"qT": qT, "kT": kT, "kgT": kgT,
                        "v65": v65, "vg65": vg65})
    return in_maps


def unpack_outputs(results):
    """Per-core oT [65, PAIRS, S_TOT] (fp16, unnormalized + Z row) ->
    full [B, S_TOT, HD] fp32, dividing by Z on the host."""
    out5 = np.empty((B, S_TOT, H, D), np.float32)
    for c in range(NCORES):
        oT = np.asarray(results[c]["oT"], dtype=np.float32)
        for p in range(PAIRS):
            b, h = divmod(c * PAIRS + p, H)
            out5[b, :, h, :] = (oT[:64, p, :] / oT[64:65, p, :]).T
    return out5.reshape(B, S_TOT, HD)


def kernel(q, k, v, num_heads, num_shots, per_g):
    assert int(num_heads) == H and int(num_shots) == NSHOT and int(per_g) == PER_G
    nc = build_program()
    in_maps = pack_inputs(np.asarray(q), np.asarray(k), np.asarray(v))
    res = run_bass_kernel_spmd(nc, in_maps, list(range(NCORES)))
    return unpack_outputs(res.results)
